# revision 5
# baseline (speedup 1.0000x reference)
"""GazeLoss Trainium2 kernel.

The reference bilinear-samples a 32x32 grid from each eye's padded bbox in
pred/target and takes mean L1 of the patch difference.  The sampling grid is
separable (x coords depend only on the grid column, y coords only on the grid
row), so sampling is linear: patch = Gy @ img @ Gx with per-batch sparse
interpolation matrices Gy (32x512, two nonzeros per row) and Gx (512x32).
Since sampling is linear, patch(pred) - patch(target) = Gy @ (pred-target) @ Gx.

Device kernel (per core, 8 batches, data parallel):
  for each (b, c) image:
    D   = pred[b,c] - target[b,c]                   (DVE, f32 -> bf16)
    FT  = D^T @ Gy'  via 16 matmuls                 (PE, psum [128x, 64r] x4 xch)
    O   = FT^T @ Gx' via 4 matmuls                  (PE, psum [64r, 64c])
    acc[:, j] = rowwise sum |O| of the two diagonal 32x32 eye blocks (DVE)
  DMA acc (64, 24) -> DRAM; host sums partials and normalizes.

Gy'/Gx' are host-built from landmarks only (the grid is constant wrt
pred/target, mirroring the reference's stop_gradient), packed bf16.
"""

import numpy as np
import ml_dtypes

EYE_SIZE = 32
PAD = 0.3
LEFT_IDX = np.arange(36, 42)
RIGHT_IDX = np.arange(42, 48)
B, C, H, W = 64, 3, 512, 512
S = EYE_SIZE
N_CORES = 8
BPC = B // N_CORES  # batches per core
N_IMG = BPC * C  # images per core

_COMPILED = None


# ---------------------------------------------------------------------------
# Host-side grid math (pure f32, mirrors the jax reference bit-for-bit-ish)
# ---------------------------------------------------------------------------

def _eye_bbox(lm, idx):
    pts = lm[:, idx, :]
    x_min = pts[:, :, 0].min(axis=1)
    x_max = pts[:, :, 0].max(axis=1)
    y_min = pts[:, :, 1].min(axis=1)
    y_max = pts[:, :, 1].max(axis=1)
    w = x_max - x_min
    h = y_max - y_min
    return (x_min - w * PAD, y_min - h * PAD, x_max + w * PAD, y_max + h * PAD)


def _grid_1d(x1, y1, x2, y2):
    # separable sample coords: px (B,S) per grid column, py (B,S) per grid row
    bx1 = np.clip(x1, 0.0, W - 1.0).astype(np.float32)
    by1 = np.clip(y1, 0.0, H - 1.0).astype(np.float32)
    bx2 = np.clip(x2, 0.0, W - 1.0).astype(np.float32)
    by2 = np.clip(y2, 0.0, H - 1.0).astype(np.float32)
    degenerate = (bx2 - bx1 < 1.0) | (by2 - by1 < 1.0)
    xn0 = bx1 / (W - 1) * np.float32(2.0) - np.float32(1.0)
    xn1 = bx2 / (W - 1) * np.float32(2.0) - np.float32(1.0)
    yn0 = by1 / (H - 1) * np.float32(2.0) - np.float32(1.0)
    yn1 = by2 / (H - 1) * np.float32(2.0) - np.float32(1.0)
    t = np.arange(S, dtype=np.float32) / np.float32(S - 1)
    xs = xn0[:, None] + (xn1 - xn0)[:, None] * t
    ys = yn0[:, None] + (yn1 - yn0)[:, None] * t
    xs[degenerate] = 0.0
    ys[degenerate] = 0.0
    px = np.clip((xs + np.float32(1.0)) * np.float32(0.5) * (W - 1), 0.0, W - 1.0)
    py = np.clip((ys + np.float32(1.0)) * np.float32(0.5) * (H - 1), 0.0, H - 1.0)
    return px.astype(np.float32), py.astype(np.float32)


def _scatter_weights(G, coord, col_base):
    # G: (B, 512, 64); coord: (B, S) sample positions along one image axis
    c0 = np.floor(coord)
    w = (coord - c0).astype(np.float32)
    i0 = np.clip(c0, 0, W - 1).astype(np.int64)
    i1 = np.clip(c0 + 1, 0, W - 1).astype(np.int64)
    b_idx = np.arange(G.shape[0])[:, None]
    j = (np.arange(S) + col_base)[None, :]
    np.add.at(G, (b_idx, i0, j), np.float32(1.0) - w)
    np.add.at(G, (b_idx, i1, j), w)


def _build_gmats(landmarks):
    # Gy[b, y, e*32+r], Gx[b, x, e*32+c]; e=0 left eye, e=1 right eye
    Gy = np.zeros((B, H, 2 * S), dtype=np.float32)
    Gx = np.zeros((B, W, 2 * S), dtype=np.float32)
    for e, idx in enumerate((LEFT_IDX, RIGHT_IDX)):
        x1, y1, x2, y2 = _eye_bbox(landmarks, idx)
        px, py = _grid_1d(x1, y1, x2, y2)
        _scatter_weights(Gx, px, e * S)
        _scatter_weights(Gy, py, e * S)
    return Gy, Gx


# ---------------------------------------------------------------------------
# Device kernel
# ---------------------------------------------------------------------------

def _build_nc():
    import concourse.bass as bass
    import concourse.mybir as mybir
    import concourse.tile as tile
    from concourse import bacc

    f32 = mybir.dt.float32
    bf16 = mybir.dt.bfloat16

    nc = bacc.Bacc("TRN2", target_bir_lowering=False, debug=False,
                   num_devices=N_CORES)
    pred = nc.dram_tensor("pred", (BPC, C, H, W), f32, kind="ExternalInput")
    target = nc.dram_tensor("target", (BPC, C, H, W), f32, kind="ExternalInput")
    # gy/gx packed (row_in_chunk, chunk, batch, 64) so the sbuf free layout is
    # chunk*512 + b*64 + j with partition = row_in_chunk
    gy = nc.dram_tensor("gy", (128, 4, BPC, 2 * S), bf16, kind="ExternalInput")
    gx = nc.dram_tensor("gx", (128, 4, BPC, 2 * S), bf16, kind="ExternalInput")
    out = nc.dram_tensor("o", (2 * S, N_IMG), f32, kind="ExternalOutput")

    with tile.TileContext(nc) as tc:
        with (
            tc.tile_pool(name="consts", bufs=1) as cpool,
            tc.tile_pool(name="img", bufs=3) as img_pool,
            tc.tile_pool(name="dtile", bufs=2) as d_pool,
            tc.tile_pool(name="ft", bufs=2) as ft_pool,
            tc.tile_pool(name="ps_ft", bufs=2, space="PSUM") as psf_pool,
            tc.tile_pool(name="ps_o", bufs=2, space="PSUM") as pso_pool,
        ):
            gy_s = cpool.tile([128, 4, BPC * 2 * S], bf16, name="gy_s")
            gx_s = cpool.tile([128, 4, BPC * 2 * S], bf16, name="gx_s")
            acc = cpool.tile([2 * S, N_IMG], f32, name="acc")
            nc.sync.dma_start(out=gy_s[:], in_=gy.ap().rearrange("p c b r -> p c (b r)"))
            nc.sync.dma_start(out=gx_s[:], in_=gx.ap().rearrange("p c b r -> p c (b r)"))

            for b in range(BPC):
                for ch in range(C):
                    j = b * C + ch
                    pt = img_pool.tile([128, 4, W], f32, tag="p")
                    tt = img_pool.tile([128, 4, W], f32, tag="t")
                    nc.sync.dma_start(
                        out=pt[:],
                        in_=pred.ap()[b, ch].rearrange("(n p) x -> p n x", p=128),
                    )
                    nc.sync.dma_start(
                        out=tt[:],
                        in_=target.ap()[b, ch].rearrange("(n p) x -> p n x", p=128),
                    )
                    d = d_pool.tile([128, 4, W], bf16, tag="d")
                    nc.vector.tensor_sub(d[:], pt[:], tt[:])

                    # FT[x, j] = sum_y D[y, x] * Gy[y, j]  (4 psum column groups)
                    ftp = psf_pool.tile([128, 4 * 2 * S], f32, tag="ftp")
                    for xch in range(4):
                        for ych in range(4):
                            nc.tensor.matmul(
                                ftp[:, xch * 64:(xch + 1) * 64],
                                d[:, ych, xch * 128:(xch + 1) * 128],
                                gy_s[:, ych, b * 64: b * 64 + 64],
                                start=(ych == 0),
                                stop=(ych == 3),
                            )
                    ft = ft_pool.tile([128, 4 * 2 * S], bf16, tag="ft")
                    nc.scalar.copy(ft[:], ftp[:])

                    # O[j, n] = sum_x FT[x, j] * Gx[x, n]
                    op = pso_pool.tile([2 * S, 2 * S], f32, tag="op")
                    for xch in range(4):
                        nc.tensor.matmul(
                            op[:],
                            ft[:, xch * 64:(xch + 1) * 64],
                            gx_s[:, xch, b * 64: b * 64 + 64],
                            start=(xch == 0),
                            stop=(xch == 3),
                        )
                    # sum |O| over the free axis of the two diagonal eye blocks
                    nc.vector.tensor_reduce(
                        acc[0:S, j:j + 1], op[0:S, 0:S],
                        axis=mybir.AxisListType.X, op=mybir.AluOpType.add,
                        apply_absolute_value=True,
                    )
                    nc.vector.tensor_reduce(
                        acc[S:2 * S, j:j + 1], op[S:2 * S, S:2 * S],
                        axis=mybir.AxisListType.X, op=mybir.AluOpType.add,
                        apply_absolute_value=True,
                    )

            nc.sync.dma_start(out=out.ap()[:, :], in_=acc[:])

    nc.compile()
    return nc


def _get_compiled():
    global _COMPILED
    if _COMPILED is None:
        _COMPILED = _build_nc()
    return _COMPILED


def _run_device(pred, target, landmarks, trace=False):
    from concourse import bass_utils

    pred = np.ascontiguousarray(np.asarray(pred, dtype=np.float32))
    target = np.ascontiguousarray(np.asarray(target, dtype=np.float32))
    landmarks = np.asarray(landmarks, dtype=np.float32)

    Gy, Gx = _build_gmats(landmarks)
    # (B, 512, 64) -> (4, 128, BPC, 64) per core, bf16
    def pack(Gfull, lo, hi):
        g = Gfull[lo:hi].reshape(BPC, 4, 128, 2 * S)
        g = np.ascontiguousarray(g.transpose(2, 1, 0, 3))
        return g.astype(ml_dtypes.bfloat16)

    in_maps = []
    for i in range(N_CORES):
        lo, hi = i * BPC, (i + 1) * BPC
        in_maps.append({
            "pred": pred[lo:hi],
            "target": target[lo:hi],
            "gy": pack(Gy, lo, hi),
            "gx": pack(Gx, lo, hi),
        })

    nc = _get_compiled()
    res = bass_utils.run_bass_kernel_spmd(
        nc, in_maps, core_ids=list(range(N_CORES)), trace=trace
    )
    total = np.float64(0.0)
    for i in range(N_CORES):
        total += res.results[i]["o"].astype(np.float64).sum()
    n = B * C * S * S
    loss = total / n / 2.0
    return np.float32(loss), res


def kernel(pred, target, landmarks):
    loss, _ = _run_device(pred, target, landmarks, trace=False)
    return loss


# revision 6
# speedup vs baseline: 2.0227x; 2.0227x over previous
"""GazeLoss Trainium2 kernel.

The reference bilinear-samples a 32x32 grid from each eye's padded bbox in
pred/target and takes mean L1 of the patch difference.  The sampling grid is
separable (x coords depend only on the grid column, y coords only on the grid
row), so sampling is linear: patch = Gy @ img @ Gx with per-batch sparse
interpolation matrices Gy (32x512, two nonzeros per row) and Gx (512x32).
Since sampling is linear, patch(pred) - patch(target) = Gy @ (pred-target) @ Gx.

Only the image rows that carry nonzero Gy weight matter: <=128 distinct rows
per batch (union over both eyes).  The device kernel gathers exactly those
rows with SWDGE dma_gather (row indices are data, so one static NEFF serves
every input), then does the interpolation as two small matmul stages.

Per core (8 batches, data parallel):
  for each batch b:
    Pg/Tg = dma_gather of the <=128 needed rows x 3 channels   (12 MiB/core
            total vs 48 MiB for a full read)
    D     = Pg - Tg                                    (DVE, f32 -> bf16)
    per channel: FT[x,j] = sum_p D[p,x] Gyc[p,j]       (PE, 4 matmuls, K=128)
                 O[j,n]  = sum_x FT[x,j] Gx[x,n]       (PE, 4 matmuls)
                 acc[:,img] = rowsum |O| of the two diagonal 32x32 eye blocks
  DMA acc (64, 24) -> DRAM; host sums partials and normalizes.

Gyc (compact row weights), Gx, and the gather indices are host-built from
landmarks only (the grid is constant wrt pred/target, mirroring the
reference's stop_gradient).
"""

import numpy as np
import ml_dtypes

EYE_SIZE = 32
PAD = 0.3
LEFT_IDX = np.arange(36, 42)
RIGHT_IDX = np.arange(42, 48)
B, C, H, W = 64, 3, 512, 512
S = EYE_SIZE
N_CORES = 8
BPC = B // N_CORES  # batches per core
N_IMG = BPC * C  # images per core
NIDX = C * 128  # gathered rows per (batch, tensor): 128 rows x 3 channels

_COMPILED = None


# ---------------------------------------------------------------------------
# Host-side grid math (pure f32, mirrors the jax reference)
# ---------------------------------------------------------------------------

def _eye_bbox(lm, idx):
    pts = lm[:, idx, :]
    x_min = pts[:, :, 0].min(axis=1)
    x_max = pts[:, :, 0].max(axis=1)
    y_min = pts[:, :, 1].min(axis=1)
    y_max = pts[:, :, 1].max(axis=1)
    w = x_max - x_min
    h = y_max - y_min
    return (x_min - w * PAD, y_min - h * PAD, x_max + w * PAD, y_max + h * PAD)


def _grid_1d(x1, y1, x2, y2):
    # separable sample coords: px (B,S) per grid column, py (B,S) per grid row
    bx1 = np.clip(x1, 0.0, W - 1.0).astype(np.float32)
    by1 = np.clip(y1, 0.0, H - 1.0).astype(np.float32)
    bx2 = np.clip(x2, 0.0, W - 1.0).astype(np.float32)
    by2 = np.clip(y2, 0.0, H - 1.0).astype(np.float32)
    degenerate = (bx2 - bx1 < 1.0) | (by2 - by1 < 1.0)
    xn0 = bx1 / (W - 1) * np.float32(2.0) - np.float32(1.0)
    xn1 = bx2 / (W - 1) * np.float32(2.0) - np.float32(1.0)
    yn0 = by1 / (H - 1) * np.float32(2.0) - np.float32(1.0)
    yn1 = by2 / (H - 1) * np.float32(2.0) - np.float32(1.0)
    t = np.arange(S, dtype=np.float32) / np.float32(S - 1)
    xs = xn0[:, None] + (xn1 - xn0)[:, None] * t
    ys = yn0[:, None] + (yn1 - yn0)[:, None] * t
    xs[degenerate] = 0.0
    ys[degenerate] = 0.0
    px = np.clip((xs + np.float32(1.0)) * np.float32(0.5) * (W - 1), 0.0, W - 1.0)
    py = np.clip((ys + np.float32(1.0)) * np.float32(0.5) * (H - 1), 0.0, H - 1.0)
    return px.astype(np.float32), py.astype(np.float32)


def _interp_pairs(coord):
    # coord (B,S) -> i0, i1 (int), w (f32): value = (1-w)*row[i0] + w*row[i1]
    c0 = np.floor(coord)
    w = (coord - c0).astype(np.float32)
    i0 = np.clip(c0, 0, W - 1).astype(np.int64)
    i1 = np.clip(c0 + 1, 0, W - 1).astype(np.int64)
    return i0, i1, w


def _build_host_tables(landmarks):
    """Per batch: compact Gy (128, 64) over the gathered-row positions,
    full Gx (512, 64), gather idx table (128, NIDX//16) int16."""
    eyes = []
    for idx in (LEFT_IDX, RIGHT_IDX):
        x1, y1, x2, y2 = _eye_bbox(landmarks, idx)
        px, py = _grid_1d(x1, y1, x2, y2)
        eyes.append((_interp_pairs(px), _interp_pairs(py)))

    Gyc = np.zeros((B, 128, 2 * S), dtype=np.float32)
    Gx = np.zeros((B, W, 2 * S), dtype=np.float32)
    idx_tab = np.zeros((B, 128, NIDX // 16), dtype=np.int16)
    for b in range(B):
        rows = set()
        for (_, (y0, y1, _)) in eyes:
            rows.update(y0[b].tolist())
            rows.update(y1[b].tolist())
        rows = sorted(rows)
        nb = len(rows)
        assert nb <= 128
        pos = {y: p for p, y in enumerate(rows)}
        for e, ((x0, x1, wx), (y0, y1, wy)) in enumerate(eyes):
            for r in range(S):
                Gyc[b, pos[y0[b, r]], e * S + r] += np.float32(1.0) - wy[b, r]
                Gyc[b, pos[y1[b, r]], e * S + r] += wy[b, r]
                Gx[b, x0[b, r], e * S + r] += np.float32(1.0) - wx[b, r]
                Gx[b, x1[b, r], e * S + r] += wx[b, r]
        rows_padded = np.zeros(128, dtype=np.int64)
        rows_padded[:nb] = rows
        # idx j = c*128 + p -> row (c, rows_padded[p]); wrapped [j%16, j//16],
        # replicated over the 8 groups of 16 partitions
        flat = (np.arange(C)[:, None] * H + rows_padded[None, :]).reshape(-1)
        wrapped = flat.reshape(NIDX // 16, 16).T.astype(np.int16)
        idx_tab[b] = np.tile(wrapped, (8, 1))
    return Gyc, Gx, idx_tab


# ---------------------------------------------------------------------------
# Device kernel
# ---------------------------------------------------------------------------

def _build_nc():
    import concourse.mybir as mybir
    import concourse.tile as tile
    from concourse import bacc
    from concourse import library_config

    f32 = mybir.dt.float32
    bf16 = mybir.dt.bfloat16
    i16 = mybir.dt.int16

    nc = bacc.Bacc("TRN2", target_bir_lowering=False, debug=False,
                   num_devices=N_CORES)
    pred = nc.dram_tensor("pred", (BPC, C, H, W), f32, kind="ExternalInput")
    target = nc.dram_tensor("target", (BPC, C, H, W), f32, kind="ExternalInput")
    gyc = nc.dram_tensor("gyc", (BPC, 128, 2 * S), bf16, kind="ExternalInput")
    # gx packed (row_in_chunk, chunk, batch, 64): sbuf free = chunk, b*64+j
    gx = nc.dram_tensor("gx", (128, 4, BPC, 2 * S), bf16, kind="ExternalInput")
    idx = nc.dram_tensor("idx", (BPC, 128, NIDX // 16), i16, kind="ExternalInput")
    out = nc.dram_tensor("o", (2 * S, N_IMG), f32, kind="ExternalOutput")

    with tile.TileContext(nc) as tc:
        with (
            tc.tile_pool(name="consts", bufs=1) as cpool,
            tc.tile_pool(name="gat", bufs=3) as gpool,
            tc.tile_pool(name="dtile", bufs=2) as d_pool,
            tc.tile_pool(name="ft", bufs=2) as ft_pool,
            tc.tile_pool(name="ps_ft", bufs=2, space="PSUM") as psf_pool,
            tc.tile_pool(name="ps_o", bufs=2, space="PSUM") as pso_pool,
        ):
            nc.gpsimd.load_library(library_config.mlp)

            idx_s = cpool.tile([128, BPC, NIDX // 16], i16, name="idx_s")
            gyc_s = cpool.tile([128, BPC, 2 * S], bf16, name="gyc_s")
            gx_s = cpool.tile([128, 4, BPC * 2 * S], bf16, name="gx_s")
            acc = cpool.tile([2 * S, N_IMG], f32, name="acc")
            nc.sync.dma_start(out=idx_s[:], in_=idx.ap().rearrange("b p s -> p b s"))
            nc.sync.dma_start(out=gyc_s[:], in_=gyc.ap().rearrange("b p r -> p b r"))
            nc.sync.dma_start(out=gx_s[:], in_=gx.ap().rearrange("p c b r -> p c (b r)"))

            for b in range(BPC):
                pg = gpool.tile([128, C, W], f32, tag="pg")
                tg = gpool.tile([128, C, W], f32, tag="tg")
                nc.gpsimd.dma_gather(
                    pg[:], pred.ap()[b].rearrange("c h w -> (c h) w"),
                    idx_s[:, b, :], NIDX, NIDX, W,
                )
                nc.gpsimd.dma_gather(
                    tg[:], target.ap()[b].rearrange("c h w -> (c h) w"),
                    idx_s[:, b, :], NIDX, NIDX, W,
                )
                d = d_pool.tile([128, C, W], bf16, tag="d")
                nc.vector.tensor_sub(d[:], pg[:], tg[:])

                for ch in range(C):
                    j = b * C + ch
                    # FT[x, j] = sum_p D[p, x] * Gyc[p, j]
                    ftp = psf_pool.tile([128, 4 * 2 * S], f32, tag="ftp")
                    for xch in range(4):
                        nc.tensor.matmul(
                            ftp[:, xch * 64:(xch + 1) * 64],
                            d[:, ch, xch * 128:(xch + 1) * 128],
                            gyc_s[:, b, :],
                            start=True, stop=True,
                        )
                    ft = ft_pool.tile([128, 4 * 2 * S], bf16, tag="ft")
                    nc.scalar.copy(ft[:], ftp[:])

                    # O[j, n] = sum_x FT[x, j] * Gx[x, n]
                    op = pso_pool.tile([2 * S, 2 * S], f32, tag="op")
                    for xch in range(4):
                        nc.tensor.matmul(
                            op[:],
                            ft[:, xch * 64:(xch + 1) * 64],
                            gx_s[:, xch, b * 64: b * 64 + 64],
                            start=(xch == 0), stop=(xch == 3),
                        )
                    nc.vector.tensor_reduce(
                        acc[0:S, j:j + 1], op[0:S, 0:S],
                        axis=mybir.AxisListType.X, op=mybir.AluOpType.add,
                        apply_absolute_value=True,
                    )
                    nc.vector.tensor_reduce(
                        acc[S:2 * S, j:j + 1], op[S:2 * S, S:2 * S],
                        axis=mybir.AxisListType.X, op=mybir.AluOpType.add,
                        apply_absolute_value=True,
                    )

            nc.sync.dma_start(out=out.ap()[:, :], in_=acc[:])

    nc.compile()
    return nc


def _get_compiled():
    global _COMPILED
    if _COMPILED is None:
        _COMPILED = _build_nc()
    return _COMPILED


def _run_device(pred, target, landmarks, trace=False):
    from concourse import bass_utils

    pred = np.ascontiguousarray(np.asarray(pred, dtype=np.float32))
    target = np.ascontiguousarray(np.asarray(target, dtype=np.float32))
    landmarks = np.asarray(landmarks, dtype=np.float32)

    Gyc, Gx, idx_tab = _build_host_tables(landmarks)
    gyc_bf = Gyc.astype(ml_dtypes.bfloat16)
    # (B, 512, 64) -> (128, 4, B, 64) bf16
    gx_packed = np.ascontiguousarray(
        Gx.reshape(B, 4, 128, 2 * S).transpose(2, 1, 0, 3)
    ).astype(ml_dtypes.bfloat16)

    in_maps = []
    for i in range(N_CORES):
        lo, hi = i * BPC, (i + 1) * BPC
        in_maps.append({
            "pred": pred[lo:hi],
            "target": target[lo:hi],
            "gyc": gyc_bf[lo:hi],
            "gx": np.ascontiguousarray(gx_packed[:, :, lo:hi]),
            "idx": idx_tab[lo:hi],
        })

    nc = _get_compiled()
    res = bass_utils.run_bass_kernel_spmd(
        nc, in_maps, core_ids=list(range(N_CORES)), trace=trace
    )
    total = np.float64(0.0)
    for i in range(N_CORES):
        total += res.results[i]["o"].astype(np.float64).sum()
    n = B * C * S * S
    loss = total / n / 2.0
    return np.float32(loss), res


def kernel(pred, target, landmarks):
    loss, _ = _run_device(pred, target, landmarks, trace=False)
    return loss
